# revision 1
# baseline (speedup 1.0000x reference)
"""AdaptiveNormalization Trainium2 kernel (8 NeuronCores, batch-parallel).

Reference computation (per batch b):
    a      = ema(x, m)                      # causal EMA over T, per (b,c)
    shift  = sum_c w_shift[c] * a[c,t]      # (b,t)
    x1     = x - shift
    bb     = ema(x1^2, m)
    scale  = sum_c exp(w_scale_log)[c] * bb[c,t]
    out    = (x1 / sqrt(scale+eps)) * w_proj[c] + b_proj[c]

Key rewrite: the EMA is a linear time-invariant filter applied identically to
every channel, so the channel reduction commutes with it:
    shift = ema(s),  s_t = sum_c w_shift[c] x[c,t]
    scale = ema(q),  q_t = u_t - 2*shift_t*v_t + shift_t^2*E
        u = sum_c e_c x^2,  v = sum_c e_c x,  E = sum_c e_c
This collapses 2*B*C length-T scans into 2*B scalar scans, computed with
hardware tensor_tensor_scan on a [blocks x 128] layout; inter-block carries
are themselves a short EMA computed by a second scan on a [1, NB] row.

T is processed in NSEG segments pipelined through three phases (stats ->
scans -> output), with scan carries chained across segments, so the
streaming phases of consecutive segments overlap on different engines.

Final output:  out = x (.) A + B  with rank-1 A = w_proj (x) inv and
rank-2 B = b_proj (x) 1 - w_proj (x) (shift*inv), produced by small bf16
PE matmuls (inv is split hi/lo bf16 for fp32-grade accuracy of A).
"""

import sys
import os

for _p in ("/opt/trn_rl_repo",):
    if _p not in sys.path:
        sys.path.insert(0, _p)

import numpy as np
import ml_dtypes
from contextlib import ExitStack

import concourse.bass as bass
import concourse.bacc as bacc
import concourse.tile as tile
from concourse import mybir
from concourse import bass_utils

MOMENTUM = 0.01
EPS = 1e-6
B, C, T_FULL = 8, 256, 16384
N_CORES = 8
BS = 128          # scan block size (elements per block, on the free dim)
NSEG = 4          # pipeline segments over T

F32 = mybir.dt.float32
F32R = mybir.dt.float32r
BF16 = mybir.dt.bfloat16
AOP = mybir.AluOpType
ACTF = mybir.ActivationFunctionType
BF = ml_dtypes.bfloat16


def _host_constants(w_shift, w_scale_log, w_proj, b_proj, T):
    """Host-side folded weights (f32 for stats, bf16 splits for A/B)."""
    m = MOMENTUM
    NB = T // NSEG // BS
    ws = w_shift.astype(np.float64)
    e = np.exp(w_scale_log.astype(np.float64))
    wp = w_proj.astype(np.float64)
    bp = b_proj.astype(np.float64)

    w_sv = np.zeros((128, 4), np.float64)
    w_sv[:, 0] = m * ws[:128]
    w_sv[:, 1] = -2.0 * m * e[:128]
    w_sv[:, 2] = m * ws[128:]
    w_sv[:, 3] = -2.0 * m * e[128:]

    w_u = np.zeros((128, 4), np.float64)
    w_u[:, 0] = m * e[:128]
    w_u[:, 2] = m * e[128:]

    # A = wp (x) inv with bf16 hi/lo compensation:
    #   A ~= wp_hi(x)inv_hi + wp_hi(x)inv_lo + wp_lo(x)inv_hi
    wp32 = wp.astype(np.float32)
    wp_hi = wp32.astype(BF)
    wp_lo = (wp32 - wp_hi.astype(np.float32)).astype(BF)
    w_a3 = np.stack([wp_hi, wp_hi, wp_lo], 0)          # (3, 256) bf16
    # B = bp (x) ones - wp (x) si   (single bf16 is enough: both terms small)
    w_b2 = np.stack([bp.astype(np.float32).astype(BF),
                     (-wp32).astype(BF)], 0)           # (2, 256) bf16

    ecol = np.full((NB, 1), m * e.sum(), np.float64)

    f = lambda a: np.ascontiguousarray(a, dtype=np.float32)
    return dict(
        w_sv=f(w_sv), w_u=f(w_u),
        w_a3=np.ascontiguousarray(w_a3), w_b2=np.ascontiguousarray(w_b2),
        e_col=f(ecol),
    )


def build_model(T=T_FULL):
    """Build the per-core Bass graph (SPMD; identical on all cores)."""
    m = MOMENTUM
    r = 1.0 - m
    r128 = r ** BS
    SEG = T // NSEG           # columns per pipeline segment
    NB = SEG // BS            # scan blocks per segment
    NGS = SEG // 512          # phase-1 groups (512 cols) per segment

    nc = bacc.Bacc("TRN2", target_bir_lowering=False, debug=False)

    # Tensors feeding fp32r matmuls are declared float32r end-to-end
    # (walrus requires fp32r matmul inputs to be produced as fp32r);
    # non-matmul consumers read them via .bitcast(F32).
    x_d = nc.dram_tensor("x", [C, T], F32R, kind="ExternalInput")
    wsv_d = nc.dram_tensor("w_sv", [128, 4], F32R, kind="ExternalInput")
    wu_d = nc.dram_tensor("w_u", [128, 4], F32R, kind="ExternalInput")
    wa3_d = nc.dram_tensor("w_a3", [3, 256], BF16, kind="ExternalInput")
    wb2_d = nc.dram_tensor("w_b2", [2, 256], BF16, kind="ExternalInput")
    ecol_d = nc.dram_tensor("e_col", [NB, 1], F32, kind="ExternalInput")
    out_d = nc.dram_tensor("out", [C, T], F32, kind="ExternalOutput")

    with tile.TileContext(nc) as tc, ExitStack() as ctx:
        consts = ctx.enter_context(tc.tile_pool(name="consts", bufs=1))
        xpool = ctx.enter_context(tc.tile_pool(name="x", bufs=2))
        scanp = ctx.enter_context(tc.tile_pool(name="scan", bufs=2))
        sqpool = ctx.enter_context(tc.tile_pool(name="sq", bufs=2))
        stpool = ctx.enter_context(tc.tile_pool(name="stage", bufs=2))
        rowp = ctx.enter_context(tc.tile_pool(name="rows", bufs=2))
        zpool = ctx.enter_context(tc.tile_pool(name="z", bufs=3))
        opool = ctx.enter_context(tc.tile_pool(name="o", bufs=3))
        dpool = ctx.enter_context(tc.tile_pool(name="dram", bufs=1, space="DRAM"))
        ps1 = ctx.enter_context(tc.tile_pool(name="ps1", bufs=2, space="PSUM"))
        ps3a = ctx.enter_context(tc.tile_pool(name="ps3a", bufs=1, space="PSUM"))
        ps3b = ctx.enter_context(tc.tile_pool(name="ps3b", bufs=1, space="PSUM"))

        # ---- constants to SBUF ----
        wsv_sb = consts.tile([128, 4], F32R)
        wu_sb = consts.tile([128, 4], F32R)
        wa3_sb = consts.tile([3, 256], BF16)
        wb2_sb = consts.tile([2, 256], BF16)
        ecol_sb = consts.tile([NB, 1], F32)
        for sb, d in ((wsv_sb, wsv_d), (wu_sb, wu_d), (wa3_sb, wa3_d),
                      (wb2_sb, wb2_d), (ecol_sb, ecol_d)):
            nc.sync.dma_start(sb[:], d[:])
        rfill = consts.tile([NB, BS], F32)
        nc.vector.memset(rfill[:], r)
        r128row = consts.tile([1, NB], F32)
        nc.vector.memset(r128row[:], r128)
        eps_sb = consts.tile([NB, 1], F32)
        nc.vector.memset(eps_sb[:], EPS)

        # cross-segment scan carries (value at very end of previous segment)
        cin = {"shift": consts.tile([1, 1], F32, tag="cin_shift",
                                    name="cin_shift"),
               "scale": consts.tile([1, 1], F32, tag="cin_scale",
                                    name="cin_scale")}
        nc.vector.memset(cin["shift"][:], 0.0)
        nc.vector.memset(cin["scale"][:], 0.0)

        def ema_scan(src, name, sig):
            """Segment EMA scan of src [NB, BS] with cross-segment carry.

            Returns the fixed-up tile; leaves the end-of-segment value in
            cin[sig] for the next segment.
            """
            loc = scanp.tile([NB, BS], F32, tag=f"{name}_loc")
            nc.vector.tensor_tensor_scan(
                loc[:], rfill[:], src[:], 0.0, AOP.mult, AOP.add)
            # block carries: c_j = r128*c_{j-1} + ylast_j, c_{-1} = cin
            yrow = scanp.tile([1, NB], F32, tag=f"{name}_yrow")
            nc.sync.dma_start(yrow[:], loc[:, BS - 1:BS])
            crow = scanp.tile([1, NB + 1], F32, tag=f"{name}_crow")
            nc.vector.tensor_copy(crow[0:1, 0:1], cin[sig][:])
            nc.vector.tensor_tensor_scan(
                crow[0:1, 1:NB + 1], r128row[:], yrow[:],
                cin[sig][0:1, 0:1], AOP.mult, AOP.add)
            # carry INTO block j is crow[j]  (crow[0] = cin)
            ccol = scanp.tile([NB, 1], F32, tag=f"{name}_ccol")
            nc.scalar.dma_start(ccol[:], crow[0:1, 0:NB])
            fixed = scanp.tile([NB, BS], F32, tag=f"{name}_fix")
            nc.vector.tensor_tensor_scan(
                fixed[:], rfill[:], src[:], ccol[:], AOP.mult, AOP.add)
            # next segment's incoming carry = last value of this segment
            nc.scalar.dma_start(cin[sig][:], fixed[NB - 1:NB, BS - 1:BS])
            return fixed

        for s in range(NSEG):
            seg0 = s * SEG

            # ---- segment x load ----
            x_sb = xpool.tile([128, 2, SEG], F32R, tag="xseg")
            for h in (0, 1):
                nc.sync.dma_start(
                    x_sb[:, h, :], x_d[h * 128:(h + 1) * 128,
                                       seg0:seg0 + SEG])

            # DRAM scratch: stats rows [stat(s,v,u,junk), group, 512] f32
            # and bf16 rows for phase 3 (inv_hi, inv_lo, si)
            rows3 = dpool.tile([4, NGS, 512], F32, tag=f"rows3_{s}")
            rowsb = dpool.tile([3, SEG], BF16, tag=f"rowsb_{s}")

            # ---- phase 1: per-t channel reductions s', v'', u' ----
            sq_holds = {}
            for g in range(NGS):
                g0 = g * 512
                if g % 2 == 0:
                    for h in (0, 1):
                        sq = sqpool.tile([128, 1024], F32R, tag=f"sq{h}")
                        xg = x_sb[:, h, g0:g0 + 1024].bitcast(F32)
                        nc.gpsimd.tensor_tensor(sq[:], xg, xg, AOP.mult)
                        sq_holds[h] = sq
                svu_ps = ps1.tile([2, 1024], F32, tag="svu")
                for h in (0, 1):
                    xsl = x_sb[:, h, g0:g0 + 512]
                    nc.tensor.matmul(
                        svu_ps[:, 0:512],
                        lhsT=wsv_sb[:, 2 * h:2 * h + 2], rhs=xsl,
                        start=(h == 0), stop=(h == 1))
                    nc.tensor.matmul(
                        svu_ps[:, 512:1024],
                        lhsT=wu_sb[:, 2 * h:2 * h + 2],
                        rhs=sq_holds[h][:, (g % 2) * 512:(g % 2) * 512 + 512],
                        start=(h == 0), stop=(h == 1))
                stage = stpool.tile([2, 1024], F32, tag="stage")
                nc.scalar.copy(stage[:], svu_ps[:])
                # stat rows in rows3: {s:0, v:1, u:2} — keep these as
                # single-partition DMAs (multi-partition sources with
                # partition count < 16 hit a descriptor-split corruption)
                nc.sync.dma_start(rows3[0:1, g, :], stage[0:1, 0:512])
                nc.sync.dma_start(rows3[1:2, g, :], stage[1:2, 0:512])
                nc.scalar.dma_start(rows3[2:3, g, :], stage[0:1, 512:1024])

            # ---- phase 2: EMA scans on [NB, BS] block layout ----
            S_s = scanp.tile([NB, BS], F32, tag="S_s")
            S_v = scanp.tile([NB, BS], F32, tag="S_v")
            S_u = scanp.tile([NB, BS], F32, tag="S_u")
            for i, St in ((0, S_s), (1, S_v), (2, S_u)):
                nc.scalar.dma_start(St[:], rows3[i:i + 1, :, :])

            shift_S = ema_scan(S_s, "shift", "shift")

            # m*q = u' + shift*v'' + shift^2 * (m*E)
            t1 = scanp.tile([NB, BS], F32, tag="t1")
            nc.vector.tensor_tensor(t1[:], shift_S[:], S_v[:], AOP.mult)
            t2 = scanp.tile([NB, BS], F32, tag="t2")
            nc.vector.tensor_tensor(t2[:], shift_S[:], shift_S[:], AOP.mult)
            q1 = scanp.tile([NB, BS], F32, tag="q1")
            nc.vector.scalar_tensor_tensor(
                q1[:], t2[:], ecol_sb[:], S_u[:], AOP.mult, AOP.add)
            qm = scanp.tile([NB, BS], F32, tag="qm")
            nc.vector.tensor_tensor(qm[:], q1[:], t1[:], AOP.add)

            scale_S = ema_scan(qm, "scale", "scale")

            # inv = 1/sqrt(scale+eps); si = shift*inv; bf16 splits for A
            sq_s = scanp.tile([NB, BS], F32, tag="sq_s")
            nc.scalar.activation(sq_s[:], scale_S[:], ACTF.Sqrt, bias=eps_sb[:])
            inv_S = scanp.tile([NB, BS], F32, tag="inv_S")
            nc.vector.reciprocal(inv_S[:], sq_s[:])
            si_S = scanp.tile([NB, BS], BF16, tag="si_S")
            nc.vector.tensor_tensor(si_S[:], shift_S[:], inv_S[:], AOP.mult)
            invh = scanp.tile([NB, BS], BF16, tag="invh")
            nc.vector.tensor_copy(invh[:], inv_S[:])
            invl = scanp.tile([NB, BS], BF16, tag="invl")
            nc.vector.tensor_tensor(invl[:], inv_S[:], invh[:], AOP.subtract)

            for i, St in ((0, invh), (1, invl), (2, si_S)):
                nc.scalar.dma_start(rowsb[i:i + 1, :], St[:])

            # ---- phase 3: out = x .* A + B ----
            ra = rowp.tile([3, SEG], BF16, tag="ra")
            rab = rowp.tile([2, SEG], BF16, tag="rab")
            nc.sync.dma_start(ra[0:1, :], rowsb[0:1, :])
            nc.sync.dma_start(ra[1:2, :], rowsb[1:2, :])
            nc.sync.dma_start(ra[2:3, :], rowsb[0:1, :])
            nc.vector.memset(rab[0:1, :], 1.0)
            nc.sync.dma_start(rab[1:2, :], rowsb[2:3, :])

            OCH = min(2048, SEG)
            for w in range(SEG // OCH):
                for h in (0, 1):
                    ot = opool.tile([128, OCH], F32, tag="ot")
                    for k in range(OCH // 1024):
                        lsl = slice(w * OCH + k * 1024,
                                    w * OCH + (k + 1) * 1024)
                        a_ps = ps3a.tile([128, 1024], F32, tag="aps")
                        b_ps = ps3b.tile([128, 1024], F32, tag="bps")
                        for j in (0, 1):
                            jsl = slice(lsl.start + j * 512,
                                        lsl.start + (j + 1) * 512)
                            nc.tensor.matmul(
                                a_ps[:, j * 512:(j + 1) * 512],
                                lhsT=wa3_sb[:, h * 128:(h + 1) * 128],
                                rhs=ra[:, jsl], start=True, stop=True)
                            nc.tensor.matmul(
                                b_ps[:, j * 512:(j + 1) * 512],
                                lhsT=wb2_sb[:, h * 128:(h + 1) * 128],
                                rhs=rab[:, jsl], start=True, stop=True)
                        z = zpool.tile([128, 1024], F32, tag="z")
                        nc.vector.tensor_tensor(
                            z[:], x_sb[:, h, lsl].bitcast(F32), a_ps[:],
                            AOP.mult)
                        nc.vector.tensor_tensor(
                            ot[:, k * 1024:(k + 1) * 1024], z[:], b_ps[:],
                            AOP.add)
                    gsl = slice(seg0 + w * OCH, seg0 + (w + 1) * OCH)
                    nc.sync.dma_start(out_d[h * 128:(h + 1) * 128, gsl],
                                      ot[:])

    nc.compile()
    return nc


_MODEL_CACHE = {}


def _get_model(T=T_FULL):
    if T not in _MODEL_CACHE:
        _MODEL_CACHE[T] = build_model(T)
    return _MODEL_CACHE[T]


def make_in_maps(x, w_shift, w_scale_log, w_proj, b_proj, T):
    """Per-core input dicts (core i gets batch i)."""
    consts = _host_constants(w_shift, w_scale_log, w_proj, b_proj, T)
    nb = x.shape[0]
    in_maps = []
    for i in range(nb):
        im = {"x": np.ascontiguousarray(x[i], dtype=np.float32)}
        im.update(consts)
        in_maps.append(im)
    return in_maps


def kernel(x, w_shift, w_scale_log, w_proj, b_proj):
    T = x.shape[-1]
    nc = _get_model(T)
    in_maps = make_in_maps(x, w_shift, w_scale_log, w_proj, b_proj, T)
    res = bass_utils.run_bass_kernel_spmd(
        nc, in_maps, core_ids=list(range(len(in_maps))))
    out = np.stack([res.results[i]["out"] for i in range(len(in_maps))], 0)
    return out.astype(np.float32)



# revision 5
# speedup vs baseline: 1.0878x; 1.0878x over previous
"""AdaptiveNormalization Trainium2 kernel (8 NeuronCores, batch-parallel).

Reference computation (per batch b):
    a      = ema(x, m)                      # causal EMA over T, per (b,c)
    shift  = sum_c w_shift[c] * a[c,t]      # (b,t)
    x1     = x - shift
    bb     = ema(x1^2, m)
    scale  = sum_c exp(w_scale_log)[c] * bb[c,t]
    out    = (x1 / sqrt(scale+eps)) * w_proj[c] + b_proj[c]

Rewrites used here:
  * The EMA is linear and channel-independent, so the channel reduction
    commutes with it:  shift = ema(s) with s_t = sum_c w_shift[c] x[c,t],
    and scale = ema(q) with q_t = u_t - 2 shift_t v_t + shift_t^2 E,
    u = sum e_c x^2, v = sum e_c x, E = sum e_c.
  * w_proj is folded into x on the host (xt = w_proj * x, bf16), so
    out = xt*inv - (w_proj*si - b_proj)  with  si = shift*inv,
    inv = 1/sqrt(scale+eps).  Stat weights are divided by w_proj (w_proj^2
    for the square stat) to compensate.
  * All I/O is bf16 (tolerance is 2e-2); stats/scans run in f32.

Implementation notes:
  * Stats are bf16 PE matmuls quadrant-packed at PSUM base partitions
    {0,32,64} so one ACT copy drains 3 groups (engine cost scales with
    free size only).
  * The per-(32-block) scan carries are computed with small PE matmuls
    (lower-triangular r^128-power matrix), avoiding per-scan DMAs.
  * inv/si rows are replicated to 128 partitions with a 7-step SBUF->SBUF
    DMA doubling cascade, keeping the phase-3 DVE ops all-bf16-SBUF
    (tensor_tensor 2x mode, tensor_scalar 4x mode).
"""

import sys
import os

for _p in ("/opt/trn_rl_repo",):
    if _p not in sys.path:
        sys.path.insert(0, _p)

import numpy as np
import ml_dtypes
from contextlib import ExitStack

import concourse.bass as bass
import concourse.bacc as bacc
import concourse.tile as tile
from concourse import mybir
from concourse import bass_utils

MOMENTUM = 0.01
EPS = 1e-6
B, C, T_FULL = 8, 256, 16384
N_CORES = 8
BS = 128          # scan block size (columns per scan block)
NSEG = 4          # pipeline segments over T

F32 = mybir.dt.float32
BF16 = mybir.dt.bfloat16
AOP = mybir.AluOpType
ACTF = mybir.ActivationFunctionType
BF = ml_dtypes.bfloat16


def _host_constants(w_shift, w_scale_log, w_proj, b_proj, T):
    m = MOMENTUM
    r = 1.0 - m
    SEG = T // NSEG
    NB = SEG // BS
    r128 = r ** BS

    ws = w_shift.astype(np.float64)
    e = np.exp(w_scale_log.astype(np.float64))
    wp = w_proj.astype(np.float64)
    bp = b_proj.astype(np.float64)

    # Stat weights, folded for xt = wp*x, zero-padded to 32 output columns
    # (so each quadrant matmul initializes a full 32-partition stripe).
    w_sv = np.zeros((128, 2, 32), np.float64)
    w_u = np.zeros((128, 2, 32), np.float64)
    for h in (0, 1):
        sl = slice(128 * h, 128 * (h + 1))
        w_sv[:, h, 0] = m * ws[sl] / wp[sl]
        w_sv[:, h, 1] = -2.0 * m * e[sl] / wp[sl]
        w_u[:, h, 0] = m * e[sl] / wp[sl] ** 2

    # per-half per-partition scalars for the D tensor_scalar
    wpb = np.zeros((128, 2, 2), np.float64)
    for h in (0, 1):
        sl = slice(128 * h, 128 * (h + 1))
        wpb[:, h, 0] = wp[sl]
        wpb[:, h, 1] = bp[sl]

    ecolm = np.full((NB, 1), m * e.sum(), np.float64)

    # scan-carry matmul constants
    mcarry = np.zeros((NB, NB), np.float64)   # lhsT[k, j] = r128^(j-1-k), k<=j-1
    for j in range(NB):
        for k in range(j):
            mcarry[k, j] = r128 ** (j - 1 - k)
    rpow = np.zeros((1, NB), np.float64)
    rpow[0, :] = r128 ** np.arange(NB)
    elast = np.zeros((NB, 1), np.float64)
    elast[NB - 1, 0] = 1.0

    f = lambda a: np.ascontiguousarray(a, dtype=np.float32)
    bf = lambda a: np.ascontiguousarray(a.astype(np.float32), dtype=BF)
    return dict(
        w_sv=bf(w_sv), w_u=bf(w_u), wpb=f(wpb), e_col=f(ecolm),
        mcarry=f(mcarry), rpow=f(rpow), elast=f(elast),
    )


def build_model(T=T_FULL):
    m = MOMENTUM
    r = 1.0 - m
    SEG = T // NSEG
    NB = SEG // BS
    NGS = SEG // 512          # 512-column stat groups per segment

    nc = bacc.Bacc("TRN2", target_bir_lowering=False, debug=False)

    x_d = nc.dram_tensor("x", [128, 2, T], BF16, kind="ExternalInput")
    wsv_d = nc.dram_tensor("w_sv", [128, 2, 32], BF16, kind="ExternalInput")
    wu_d = nc.dram_tensor("w_u", [128, 2, 32], BF16, kind="ExternalInput")
    wpb_d = nc.dram_tensor("wpb", [128, 2, 2], F32, kind="ExternalInput")
    ecol_d = nc.dram_tensor("e_col", [NB, 1], F32, kind="ExternalInput")
    mcar_d = nc.dram_tensor("mcarry", [NB, NB], F32, kind="ExternalInput")
    rpow_d = nc.dram_tensor("rpow", [1, NB], F32, kind="ExternalInput")
    elast_d = nc.dram_tensor("elast", [NB, 1], F32, kind="ExternalInput")
    out_d = nc.dram_tensor("out", [128, 2, T], BF16, kind="ExternalOutput")

    with tile.TileContext(nc) as tc, ExitStack() as ctx:
        consts = ctx.enter_context(tc.tile_pool(name="consts", bufs=1))
        xpool = ctx.enter_context(tc.tile_pool(name="x", bufs=2))
        sqpool = ctx.enter_context(tc.tile_pool(name="sq", bufs=2))
        stpool = ctx.enter_context(tc.tile_pool(name="stage", bufs=2))
        scanp = ctx.enter_context(tc.tile_pool(name="scan", bufs=2))
        bcpool = ctx.enter_context(tc.tile_pool(name="bc", bufs=2))
        zpool = ctx.enter_context(tc.tile_pool(name="z", bufs=2))
        dpool = ctx.enter_context(tc.tile_pool(name="d", bufs=2))
        opool = ctx.enter_context(tc.tile_pool(name="o", bufs=2))
        ps_stat = ctx.enter_context(tc.tile_pool(name="pstat", bufs=2,
                                                 space="PSUM"))
        ps_car = ctx.enter_context(tc.tile_pool(name="pcar", bufs=2,
                                                space="PSUM"))

        # ---- constants ----
        wsv_sb = consts.tile([128, 2, 32], BF16)
        wu_sb = consts.tile([128, 2, 32], BF16)
        wpb_sb = consts.tile([128, 2, 2], F32)
        ecol_sb = consts.tile([NB, 1], F32)
        mcar_sb = consts.tile([NB, NB], F32)
        rpow_sb = consts.tile([1, NB], F32)
        elast_sb = consts.tile([NB, 1], F32)
        for sb, d in ((wsv_sb, wsv_d), (wu_sb, wu_d), (wpb_sb, wpb_d),
                      (ecol_sb, ecol_d), (mcar_sb, mcar_d),
                      (rpow_sb, rpow_d), (elast_sb, elast_d)):
            nc.sync.dma_start(sb[:], d[:])
        rfill = consts.tile([NB, BS], F32)
        nc.vector.memset(rfill[:], r)
        eps_sb = consts.tile([NB, 1], F32)
        nc.vector.memset(eps_sb[:], EPS)
        cin = {"shift": consts.tile([1, 1], F32, name="cin_shift"),
               "scale": consts.tile([1, 1], F32, name="cin_scale")}
        nc.vector.memset(cin["shift"][:], 0.0)
        nc.vector.memset(cin["scale"][:], 0.0)

        def ema_scan(src, sig, ccol4, ccol_i, cine_i):
            """Blockwise EMA scan of src [NB, BS] with PE-matmul carries.
            Chains across segments through cin[sig]."""
            loc = scanp.tile([NB, BS], F32, tag=f"{sig}_loc")
            nc.vector.tensor_tensor_scan(
                loc[:], rfill[:], src[:], 0.0, AOP.mult, AOP.add)
            # carry into block j: P_j = sum_{k<j} r128^{j-1-k} y_k + r128^j cin
            nc.tensor.matmul(ccol4[0:NB, ccol_i:ccol_i + 1],
                             lhsT=mcar_sb[:], rhs=loc[:, BS - 1:BS],
                             start=True, stop=False)
            nc.tensor.matmul(ccol4[0:NB, ccol_i:ccol_i + 1],
                             lhsT=rpow_sb[:], rhs=cin[sig][:],
                             start=False, stop=True)
            fixed = scanp.tile([NB, BS], F32, tag=f"{sig}_fix")
            nc.vector.tensor_tensor_scan(
                fixed[:], rfill[:], src[:], ccol4[0:NB, ccol_i:ccol_i + 1],
                AOP.mult, AOP.add)
            # next segment's carry-in = last value of this segment
            nc.tensor.matmul(ccol4[0:1, cine_i:cine_i + 1],
                             lhsT=elast_sb[:], rhs=fixed[:, BS - 1:BS],
                             start=True, stop=True)
            nc.vector.tensor_copy(cin[sig][:], ccol4[0:1, cine_i:cine_i + 1])
            return fixed

        # round-robin small-DMA issuers to spread queue load
        dmaq = [nc.sync, nc.scalar]

        for s in range(NSEG):
            seg0 = s * SEG

            # ---- x load ----
            xs = xpool.tile([128, 2, SEG], BF16, tag="x")
            for h in (0, 1):
                nc.sync.dma_start(xs[:, h, :], x_d[:, h, seg0:seg0 + SEG])

            # ---- phase 1: stats (s', v', u' per 512-col group) ----
            S_s = scanp.tile([NB, BS], F32, tag="S_s")
            S_v = scanp.tile([NB, BS], F32, tag="S_v")
            S_u = scanp.tile([NB, BS], F32, tag="S_u")
            g0 = 0
            di = 0
            while g0 < NGS:
                ng = min(3, NGS - g0)
                pp = 32 * ng
                sv_ps = ps_stat.tile([96, 512], F32, tag="sv")
                u_ps = ps_stat.tile([96, 512], F32, tag="u")
                sq = sqpool.tile([128, 2, 512 * ng], BF16, tag="sq")
                for gi in range(ng):
                    g = g0 + gi
                    cols = slice(512 * g, 512 * (g + 1))
                    qb = 32 * gi
                    qs = slice(512 * gi, 512 * (gi + 1))
                    for h in (0, 1):
                        nc.scalar.activation(sq[:, h, qs], xs[:, h, cols],
                                             ACTF.Square)
                    for h in (0, 1):
                        nc.tensor.matmul(sv_ps[qb:qb + 32, :],
                                         lhsT=wsv_sb[:, h, :],
                                         rhs=xs[:, h, cols],
                                         start=(h == 0), stop=(h == 1))
                        nc.tensor.matmul(u_ps[qb:qb + 32, :],
                                         lhsT=wu_sb[:, h, :],
                                         rhs=sq[:, h, qs],
                                         start=(h == 0), stop=(h == 1))
                stage_sv = stpool.tile([96, 512], F32, tag="ssv")
                stage_u = stpool.tile([96, 512], F32, tag="su")
                nc.scalar.copy(stage_sv[0:pp, :], sv_ps[0:pp, :])
                nc.scalar.copy(stage_u[0:pp, :], u_ps[0:pp, :])
                for gi in range(ng):
                    g = g0 + gi
                    bsl = slice(4 * g, 4 * g + 4)
                    dmaq[di % 2].dma_start(S_s[bsl, :],
                                           stage_sv[32 * gi:32 * gi + 1, :])
                    dmaq[(di + 1) % 2].dma_start(S_v[bsl, :],
                                                 stage_sv[32 * gi + 1:32 * gi + 2, :])
                    dmaq[di % 2].dma_start(S_u[bsl, :],
                                                 stage_u[32 * gi:32 * gi + 1, :])
                    di += 1
                g0 += ng

            # ---- phase 2: scans ----
            ccol4 = ps_car.tile([NB, 4], F32, tag="ccol")
            shift_S = ema_scan(S_s, "shift", ccol4, 0, 2)

            t1 = scanp.tile([NB, BS], F32, tag="t1")
            nc.vector.tensor_tensor(t1[:], shift_S[:], S_v[:], AOP.mult)
            t2 = scanp.tile([NB, BS], F32, tag="t2")
            nc.vector.tensor_tensor(t2[:], shift_S[:], shift_S[:], AOP.mult)
            q1 = scanp.tile([NB, BS], F32, tag="q1")
            nc.vector.scalar_tensor_tensor(
                q1[:], t2[:], ecol_sb[:], S_u[:], AOP.mult, AOP.add)
            qm = scanp.tile([NB, BS], F32, tag="qm")
            nc.vector.tensor_tensor(qm[:], q1[:], t1[:], AOP.add)

            scale_S = ema_scan(qm, "scale", ccol4, 1, 3)

            sq_s = scanp.tile([NB, BS], F32, tag="sq_s")
            nc.scalar.activation(sq_s[:], scale_S[:], ACTF.Sqrt,
                                 bias=eps_sb[:])
            inv_S = scanp.tile([NB, BS], F32, tag="inv_S")
            nc.vector.reciprocal_approx_fast(inv_S[:], sq_s[:])
            sib = scanp.tile([NB, BS], BF16, tag="sib")
            nc.vector.tensor_tensor(sib[:], shift_S[:], inv_S[:], AOP.mult)
            invb = scanp.tile([NB, BS], BF16, tag="invb")
            nc.vector.tensor_copy(invb[:], inv_S[:])

            # ---- broadcast rows to 128 partitions (doubling cascade) ----
            bc = bcpool.tile([128, 2 * SEG], BF16, tag="bc")
            nc.scalar.dma_start(bc[0:1, 0:SEG], invb[:])
            nc.scalar.dma_start(bc[0:1, SEG:2 * SEG], sib[:])
            k = 1
            while k < 128:
                nc.sync.dma_start(bc[k:2 * k, :], bc[0:k, :])
                k *= 2

            # ---- phase 3: out = xt*inv - (wp*si - bp) ----
            bcinv = bass.AP(bc[:, 0:SEG].tensor, bc[:, 0:SEG].offset,
                            [list(bc[:, 0:SEG].ap[0]), [0, 2],
                             list(bc[:, 0:SEG].ap[1])])
            z = zpool.tile([128, 2, SEG], BF16, tag="z")
            nc.vector.tensor_tensor(z[:], xs[:], bcinv, AOP.mult)
            D = dpool.tile([128, 2, SEG], BF16, tag="D")
            for h in (0, 1):
                nc.vector.tensor_scalar(D[:, h, :], bc[:, SEG:2 * SEG],
                                        wpb_sb[:, h, 0:1], wpb_sb[:, h, 1:2],
                                        AOP.mult, AOP.subtract)
            o = opool.tile([128, 2, SEG], BF16, tag="o")
            nc.vector.tensor_tensor(o[:], z[:], D[:], AOP.subtract)
            for h in (0, 1):
                nc.sync.dma_start(out_d[:, h, seg0:seg0 + SEG], o[:, h, :])

    nc.compile()
    return nc


_MODEL_CACHE = {}


def _get_model(T=T_FULL):
    if T not in _MODEL_CACHE:
        _MODEL_CACHE[T] = build_model(T)
    return _MODEL_CACHE[T]


def make_in_maps(x, w_shift, w_scale_log, w_proj, b_proj, T):
    """Per-core input dicts (core i gets batch i)."""
    consts = _host_constants(w_shift, w_scale_log, w_proj, b_proj, T)
    nb = x.shape[0]
    wp = w_proj.astype(np.float32)
    in_maps = []
    for i in range(nb):
        xt = (x[i].astype(np.float32) * wp[:, None]).astype(BF)
        xt = np.ascontiguousarray(
            np.stack([xt[:128], xt[128:]], axis=1))      # [128, 2, T]
        im = {"x": xt}
        im.update(consts)
        in_maps.append(im)
    return in_maps


def kernel(x, w_shift, w_scale_log, w_proj, b_proj):
    T = x.shape[-1]
    nc = _get_model(T)
    in_maps = make_in_maps(x, w_shift, w_scale_log, w_proj, b_proj, T)
    res = bass_utils.run_bass_kernel_spmd(
        nc, in_maps, core_ids=list(range(len(in_maps))))
    outs = []
    for i in range(len(in_maps)):
        o = np.asarray(res.results[i]["out"])            # [128, 2, T] bf16
        outs.append(np.concatenate([o[:, 0, :], o[:, 1, :]], axis=0))
    return np.stack(outs, 0).astype(np.float32)


# revision 10
# speedup vs baseline: 1.2408x; 1.1407x over previous
"""AdaptiveNormalization Trainium2 kernel (8 NeuronCores, batch-parallel).

Reference computation (per batch b):
    a      = ema(x, m)                      # causal EMA over T, per (b,c)
    shift  = sum_c w_shift[c] * a[c,t]      # (b,t)
    x1     = x - shift
    bb     = ema(x1^2, m)
    scale  = sum_c exp(w_scale_log)[c] * bb[c,t]
    out    = (x1 / sqrt(scale+eps)) * w_proj[c] + b_proj[c]

Rewrites used here:
  * The EMA is linear and channel-independent, so the channel reduction
    commutes with it:  shift = ema(s) with s_t = sum_c w_shift[c] x[c,t],
    and scale = ema(q) with q_t = u_t - 2 shift_t v_t + shift_t^2 E,
    u = sum e_c x^2, v = sum e_c x, E = sum e_c.
  * w_proj is folded into x on the host (xt = w_proj * x, bf16), so
    out = xt*inv - (w_proj*si - b_proj)  with  si = shift*inv,
    inv = 1/sqrt(scale+eps).  Stat weights are divided by w_proj (w_proj^2
    for the square stat) to compensate.
  * All I/O is bf16 (tolerance is 2e-2); stats/scans run in f32.

Implementation notes:
  * Stats are bf16 PE matmuls quadrant-packed at PSUM base partitions
    {0,32,64} so one ACT copy drains 3 groups (engine cost scales with
    free size only).
  * The per-(32-block) scan carries are computed with small PE matmuls
    (lower-triangular r^128-power matrix), avoiding per-scan DMAs.
  * inv/si rows are replicated to 128 partitions with a 7-step SBUF->SBUF
    DMA doubling cascade, keeping the phase-3 DVE ops all-bf16-SBUF
    (tensor_tensor 2x mode, tensor_scalar 4x mode).
"""

import sys
import os

for _p in ("/opt/trn_rl_repo",):
    if _p not in sys.path:
        sys.path.insert(0, _p)

import numpy as np
import ml_dtypes
from contextlib import ExitStack

import concourse.bass as bass
import concourse.bacc as bacc
import concourse.tile as tile
from concourse import mybir
from concourse import bass_utils

MOMENTUM = 0.01
EPS = 1e-6
B, C, T_FULL = 8, 256, 16384
N_CORES = 8
BS = 128          # scan block size (columns per scan block)
NSEG = 4          # pipeline segments over T

F32 = mybir.dt.float32
BF16 = mybir.dt.bfloat16
AOP = mybir.AluOpType
ACTF = mybir.ActivationFunctionType
BF = ml_dtypes.bfloat16


def _host_constants(w_shift, w_scale_log, w_proj, b_proj, T):
    m = MOMENTUM
    r = 1.0 - m
    SEG = T // NSEG
    NB = SEG // BS
    r128 = r ** BS

    ws = w_shift.astype(np.float64)
    e = np.exp(w_scale_log.astype(np.float64))
    wp = w_proj.astype(np.float64)
    bp = b_proj.astype(np.float64)

    # Stat weights, folded for xt = wp*x, zero-padded to 32 output columns
    # (so each quadrant matmul initializes a full 32-partition stripe).
    # The u matmul accumulates into the same PSUM stripe as s/v with its
    # stat at column 2, so one stripe ends up holding [s, v, u, 0...].
    w_sv = np.zeros((128, 2, 32), np.float64)
    w_u = np.zeros((128, 2, 32), np.float64)
    for h in (0, 1):
        sl = slice(128 * h, 128 * (h + 1))
        w_sv[:, h, 0] = m * ws[sl] / wp[sl]
        w_sv[:, h, 1] = -2.0 * m * e[sl] / wp[sl]
        w_u[:, h, 2] = m * e[sl] / wp[sl] ** 2

    # per-half per-partition scalars for the D tensor_scalar
    wpb = np.zeros((128, 2, 2), np.float64)
    for h in (0, 1):
        sl = slice(128 * h, 128 * (h + 1))
        wpb[:, h, 0] = wp[sl]
        wpb[:, h, 1] = bp[sl]

    ecolm = np.full((NB, 1), m * e.sum(), np.float64)

    # scan-carry matmul constants
    mcarry = np.zeros((NB, NB), np.float64)   # lhsT[k, j] = r128^(j-1-k), k<=j-1
    for j in range(NB):
        for k in range(j):
            mcarry[k, j] = r128 ** (j - 1 - k)
    rpow = np.zeros((1, NB), np.float64)
    rpow[0, :] = r128 ** np.arange(NB)
    elast = np.zeros((NB, 1), np.float64)
    elast[NB - 1, 0] = 1.0

    f = lambda a: np.ascontiguousarray(a, dtype=np.float32)
    bf = lambda a: np.ascontiguousarray(a.astype(np.float32), dtype=BF)
    return dict(
        w_sv=bf(w_sv), w_u=bf(w_u), wpb=f(wpb), e_col=f(ecolm),
        mcarry=f(mcarry), rpow=f(rpow), elast=f(elast),
    )


def build_model(T=T_FULL):
    m = MOMENTUM
    r = 1.0 - m
    SEG = T // NSEG
    NB = SEG // BS
    NGS = SEG // 512          # 512-column stat groups per segment

    nc = bacc.Bacc("TRN2", target_bir_lowering=False, debug=False)

    x_d = nc.dram_tensor("x", [128, 2, T], BF16, kind="ExternalInput")
    wsv_d = nc.dram_tensor("w_sv", [128, 2, 32], BF16, kind="ExternalInput")
    wu_d = nc.dram_tensor("w_u", [128, 2, 32], BF16, kind="ExternalInput")
    wpb_d = nc.dram_tensor("wpb", [128, 2, 2], F32, kind="ExternalInput")
    ecol_d = nc.dram_tensor("e_col", [NB, 1], F32, kind="ExternalInput")
    mcar_d = nc.dram_tensor("mcarry", [NB, NB], F32, kind="ExternalInput")
    rpow_d = nc.dram_tensor("rpow", [1, NB], F32, kind="ExternalInput")
    elast_d = nc.dram_tensor("elast", [NB, 1], F32, kind="ExternalInput")
    out_d = nc.dram_tensor("out", [128, 2, T], BF16, kind="ExternalOutput")

    with tile.TileContext(nc) as tc, ExitStack() as ctx:
        consts = ctx.enter_context(tc.tile_pool(name="consts", bufs=1))
        xpool = ctx.enter_context(tc.tile_pool(name="x", bufs=2))
        sqpool = ctx.enter_context(tc.tile_pool(name="sq", bufs=2))
        stpool = ctx.enter_context(tc.tile_pool(name="stage", bufs=2))
        scanp = ctx.enter_context(tc.tile_pool(name="scan", bufs=2))
        bcpool = ctx.enter_context(tc.tile_pool(name="bc", bufs=2))
        zpool = ctx.enter_context(tc.tile_pool(name="z", bufs=2))
        dpool = ctx.enter_context(tc.tile_pool(name="d", bufs=2))
        opool = ctx.enter_context(tc.tile_pool(name="o", bufs=2))
        ps_stat = ctx.enter_context(tc.tile_pool(name="pstat", bufs=2,
                                                 space="PSUM"))
        ps_car = ctx.enter_context(tc.tile_pool(name="pcar", bufs=2,
                                                space="PSUM"))

        # ---- constants ----
        wsv_sb = consts.tile([128, 2, 32], BF16)
        wu_sb = consts.tile([128, 2, 32], BF16)
        wpb_sb = consts.tile([128, 2, 2], F32)
        ecol_sb = consts.tile([NB, 1], F32)
        mcar_sb = consts.tile([NB, NB], F32)
        rpow_sb = consts.tile([1, NB], F32)
        elast_sb = consts.tile([NB, 1], F32)
        for sb, d in ((wsv_sb, wsv_d), (wu_sb, wu_d), (wpb_sb, wpb_d),
                      (ecol_sb, ecol_d), (mcar_sb, mcar_d),
                      (rpow_sb, rpow_d), (elast_sb, elast_d)):
            nc.sync.dma_start(sb[:], d[:])
        rfill = consts.tile([NB, BS], F32)
        nc.vector.memset(rfill[:], r)
        eps_sb = consts.tile([NB, 1], F32)
        nc.vector.memset(eps_sb[:], EPS)
        cin = {"shift": consts.tile([1, 1], F32, name="cin_shift"),
               "scale": consts.tile([1, 1], F32, name="cin_scale")}
        nc.vector.memset(cin["shift"][:], 0.0)
        nc.vector.memset(cin["scale"][:], 0.0)

        def ema_scan(src, sig, ccol4, ccol_i, cine_i):
            """Blockwise EMA scan of src [NB, BS] with PE-matmul carries.
            Chains across segments through cin[sig]."""
            loc = scanp.tile([NB, BS], F32, tag=f"{sig}_loc")
            nc.vector.tensor_tensor_scan(
                loc[:], rfill[:], src[:], 0.0, AOP.mult, AOP.add)
            # carry into block j: P_j = sum_{k<j} r128^{j-1-k} y_k + r128^j cin
            nc.tensor.matmul(ccol4[0:NB, ccol_i:ccol_i + 1],
                             lhsT=mcar_sb[:], rhs=loc[:, BS - 1:BS],
                             start=True, stop=False)
            nc.tensor.matmul(ccol4[0:NB, ccol_i:ccol_i + 1],
                             lhsT=rpow_sb[:], rhs=cin[sig][:],
                             start=False, stop=True)
            fixed = scanp.tile([NB, BS], F32, tag=f"{sig}_fix")
            nc.vector.tensor_tensor_scan(
                fixed[:], rfill[:], src[:], ccol4[0:NB, ccol_i:ccol_i + 1],
                AOP.mult, AOP.add)
            # next segment's carry-in = last value of this segment
            nc.tensor.matmul(ccol4[0:1, cine_i:cine_i + 1],
                             lhsT=elast_sb[:], rhs=fixed[:, BS - 1:BS],
                             start=True, stop=True)
            nc.vector.tensor_copy(cin[sig][:], ccol4[0:1, cine_i:cine_i + 1])
            return fixed

        GW = 1024                 # stat group width (columns)
        NG = SEG // GW            # stat groups per segment
        NBG = GW // BS            # scan blocks per group

        for s in range(NSEG):
            seg0 = s * SEG

            # ---- x load (one DMA per segment, sync queue = big I/O only) ----
            xs = xpool.tile([128, 2, SEG], BF16, tag="x")
            nc.sync.dma_start(xs[:], x_d[:, :, seg0:seg0 + SEG])

            # ---- phase 1: stats; groups of GW cols, 3 per PSUM tile at
            # quadrant bases {0,32,64}; u accumulates into the same stripe ----
            S_s = scanp.tile([NB, BS], F32, tag="S_s")
            S_v = scanp.tile([NB, BS], F32, tag="S_v")
            S_u = scanp.tile([NB, BS], F32, tag="S_u")
            g0 = 0
            while g0 < NG:
                ng = min(3, NG - g0)
                pp = 32 * ng
                svu_ps = ps_stat.tile([96, GW], F32, tag="svu")
                for gi in range(ng):
                    g = g0 + gi
                    cols = slice(GW * g, GW * (g + 1))
                    qb = 32 * gi
                    sq = sqpool.tile([128, 2, GW], BF16, tag="sq")
                    for h in (0, 1):
                        nc.scalar.activation(sq[:, h, :], xs[:, h, cols],
                                             ACTF.Square)
                    for j in (0, 1):
                        jsl = slice(512 * j, 512 * (j + 1))
                        jc = slice(cols.start + 512 * j,
                                   cols.start + 512 * (j + 1))
                        nc.tensor.matmul(svu_ps[qb:qb + 32, jsl],
                                         lhsT=wsv_sb[:, 0, :],
                                         rhs=xs[:, 0, jc],
                                         start=True, stop=False)
                        nc.tensor.matmul(svu_ps[qb:qb + 32, jsl],
                                         lhsT=wsv_sb[:, 1, :],
                                         rhs=xs[:, 1, jc],
                                         start=False, stop=False)
                        nc.tensor.matmul(svu_ps[qb:qb + 32, jsl],
                                         lhsT=wu_sb[:, 0, :],
                                         rhs=sq[:, 0, jsl],
                                         start=False, stop=False)
                        nc.tensor.matmul(svu_ps[qb:qb + 32, jsl],
                                         lhsT=wu_sb[:, 1, :],
                                         rhs=sq[:, 1, jsl],
                                         start=False, stop=True)
                stage = stpool.tile([96, GW], F32, tag="stg")
                nc.scalar.copy(stage[0:pp, :], svu_ps[0:pp, :])
                # scatter stat rows to scan-block layout (gpsimd SWDGE queue)
                for gi in range(ng):
                    g = g0 + gi
                    bsl = slice(NBG * g, NBG * (g + 1))
                    nc.gpsimd.dma_start(S_s[bsl, :],
                                        stage[32 * gi + 0:32 * gi + 1, :])
                    nc.gpsimd.dma_start(S_v[bsl, :],
                                        stage[32 * gi + 1:32 * gi + 2, :])
                    nc.gpsimd.dma_start(S_u[bsl, :],
                                        stage[32 * gi + 2:32 * gi + 3, :])
                g0 += ng

            # ---- phase 2: scans ----
            ccol4 = ps_car.tile([NB, 4], F32, tag="ccol")
            shift_S = ema_scan(S_s, "shift", ccol4, 0, 2)

            t1 = scanp.tile([NB, BS], F32, tag="t1")
            nc.vector.tensor_tensor(t1[:], shift_S[:], S_v[:], AOP.mult)
            t2 = scanp.tile([NB, BS], F32, tag="t2")
            nc.vector.tensor_tensor(t2[:], shift_S[:], shift_S[:], AOP.mult)
            q1 = scanp.tile([NB, BS], F32, tag="q1")
            nc.vector.scalar_tensor_tensor(
                q1[:], t2[:], ecol_sb[:], S_u[:], AOP.mult, AOP.add)
            qm = scanp.tile([NB, BS], F32, tag="qm")
            nc.vector.tensor_tensor(qm[:], q1[:], t1[:], AOP.add)

            scale_S = ema_scan(qm, "scale", ccol4, 1, 3)

            sq_s = scanp.tile([NB, BS], F32, tag="sq_s")
            nc.scalar.activation(sq_s[:], scale_S[:], ACTF.Sqrt,
                                 bias=eps_sb[:])
            inv_S = scanp.tile([NB, BS], F32, tag="inv_S")
            nc.vector.reciprocal_approx_fast(inv_S[:], sq_s[:])
            sib = scanp.tile([NB, BS], BF16, tag="sib")
            nc.vector.tensor_tensor(sib[:], shift_S[:], inv_S[:], AOP.mult)
            invb = scanp.tile([NB, BS], BF16, tag="invb")
            nc.vector.tensor_copy(invb[:], inv_S[:])

            # ---- broadcast rows to 128 partitions (doubling cascade) ----
            bc = bcpool.tile([128, 2 * SEG], BF16, tag="bc")
            nc.scalar.dma_start(bc[0:1, 0:SEG], invb[:])
            nc.scalar.dma_start(bc[0:1, SEG:2 * SEG], sib[:])
            k = 1
            while k < 128:
                nc.scalar.dma_start(bc[k:2 * k, :], bc[0:k, :])
                k *= 2

            # ---- phase 3: out = xt*inv - (wp*si - bp) ----
            bcinv = bass.AP(bc[:, 0:SEG].tensor, bc[:, 0:SEG].offset,
                            [list(bc[:, 0:SEG].ap[0]), [0, 2],
                             list(bc[:, 0:SEG].ap[1])])
            z = zpool.tile([128, 2, SEG], BF16, tag="z")
            nc.vector.tensor_tensor(z[:], xs[:], bcinv, AOP.mult)
            o = opool.tile([128, 2, SEG], BF16, tag="o")
            for h in (0, 1):
                D = dpool.tile([128, SEG], BF16, tag="D")
                nc.vector.tensor_scalar(D[:], bc[:, SEG:2 * SEG],
                                        wpb_sb[:, h, 0:1], wpb_sb[:, h, 1:2],
                                        AOP.mult, AOP.subtract)
                nc.vector.tensor_tensor(o[:, h, :], z[:, h, :], D[:],
                                        AOP.subtract)
            nc.sync.dma_start(out_d[:, :, seg0:seg0 + SEG], o[:])

    nc.compile()
    return nc


_MODEL_CACHE = {}


def _get_model(T=T_FULL):
    if T not in _MODEL_CACHE:
        _MODEL_CACHE[T] = build_model(T)
    return _MODEL_CACHE[T]


def make_in_maps(x, w_shift, w_scale_log, w_proj, b_proj, T):
    """Per-core input dicts (core i gets batch i)."""
    consts = _host_constants(w_shift, w_scale_log, w_proj, b_proj, T)
    nb = x.shape[0]
    wp = w_proj.astype(np.float32)
    in_maps = []
    for i in range(nb):
        xt = (x[i].astype(np.float32) * wp[:, None]).astype(BF)
        xt = np.ascontiguousarray(
            np.stack([xt[:128], xt[128:]], axis=1))      # [128, 2, T]
        im = {"x": xt}
        im.update(consts)
        in_maps.append(im)
    return in_maps


def kernel(x, w_shift, w_scale_log, w_proj, b_proj):
    T = x.shape[-1]
    nc = _get_model(T)
    in_maps = make_in_maps(x, w_shift, w_scale_log, w_proj, b_proj, T)
    res = bass_utils.run_bass_kernel_spmd(
        nc, in_maps, core_ids=list(range(len(in_maps))))
    outs = []
    for i in range(len(in_maps)):
        o = np.asarray(res.results[i]["out"])            # [128, 2, T] bf16
        outs.append(np.concatenate([o[:, 0, :], o[:, 1, :]], axis=0))
    return np.stack(outs, 0).astype(np.float32)
